# revision 1
# baseline (speedup 1.0000x reference)
"""ConditionalSigKerMMD discriminator loss on 8 TRN2 NeuronCores.

Strategy (self-contained, hardcoded for x,y,z of shape (64,33,8) fp32):
  - 4 signature-kernel Grams (K=rbf(x,x), Lgen=lin(y,y), Ltrue=lin(z,z),
    Lmix=lin(y,z)), each 64x64 pairs. Pair dim sharded 8 ways by row
    block (8 rows/core -> 512 pairs/gram/core, 2048 pairs/core total).
  - Static Gram via PE matmul in (n,s)x(m,t) layout, bounced through DRAM
    into pair-major layout [pair, (s,t)].
  - Goursat PDE solved as 64 row recurrences using the DVE
    tensor_tensor_scan: state = C1*state + D, with per-block reset slots.
  - K shards AllGathered; (K+I)^-1 via Newton-Schulz (replicated);
    E = B - B^2; partial trace tr(E*Lsum) per core; host sums 8 scalars.
"""
import numpy as np

N_CORES = 8
N = 64           # pairs per gram row/col
S = 33           # path length
D = 8            # path dim
NL = 8           # rows (n values) per core
PQ = 64          # fine grid size (S-1)*2
CC = 32          # coarse coefficient grid
GRID = S * S     # 1089
SQRT192INV = 1.0 / np.sqrt(192.0)
NS_ITERS = 14

_CACHE = {}


def _build():
    import concourse.bass as bass
    import concourse.mybir as mybir
    import concourse.tile as tile
    import concourse.bacc as bacc

    f32 = mybir.dt.float32
    AX = mybir.AxisListType
    OP = mybir.AluOpType
    AF = mybir.ActivationFunctionType

    nc = bacc.Bacc("TRN2", target_bir_lowering=False, debug=False,
                   num_devices=N_CORES)

    # ---- I/O ----
    xf = nc.dram_tensor("xf", [N, S, D], f32, kind="ExternalInput").ap()
    yf = nc.dram_tensor("yf", [N, S, D], f32, kind="ExternalInput").ap()
    zf = nc.dram_tensor("zf", [N, S, D], f32, kind="ExternalInput").ap()
    xc = nc.dram_tensor("xc", [NL, S, D], f32, kind="ExternalInput").ap()
    yc = nc.dram_tensor("yc", [NL, S, D], f32, kind="ExternalInput").ap()
    zc = nc.dram_tensor("zc", [NL, S, D], f32, kind="ExternalInput").ap()
    eye_d = nc.dram_tensor("eye", [N, N], f32, kind="ExternalInput").ap()
    sel_d = nc.dram_tensor("sel", [N, NL], f32, kind="ExternalInput").ap()
    shf_d = nc.dram_tensor("shf", [128, N], f32, kind="ExternalInput").ap()
    out_d = nc.dram_tensor("out", [1, 1], f32, kind="ExternalOutput").ap()

    # ---- internal DRAM ----
    gdr = [nc.dram_tensor(f"gram{g}", [NL, N, S, S], f32).ap() for g in range(4)]
    x2c_dr = nc.dram_tensor("x2c", [NL, S], f32).ap()
    x2f_dr = nc.dram_tensor("x2f", [N, S], f32).ap()
    ksh_dr = nc.dram_tensor("ksh", [4, 128], f32).ap()
    kall_dr = nc.dram_tensor("kall", [N_CORES, 4, 2, N], f32,
                             addr_space="Shared").ap()

    # matmul m-chunks (PSUM bank <=512 fp32, m-aligned for scatter APs)
    MCH = [(0, 15), (15, 15), (30, 15), (45, 15), (60, 4)]

    with tile.TileContext(nc) as tc:
        with (
            tc.tile_pool(name="cst", bufs=1) as cst,
            tc.tile_pool(name="lda", bufs=1) as lda,
            tc.tile_pool(name="mmp", bufs=4, space="PSUM") as mmp,
            tc.tile_pool(name="nsp", bufs=2, space="PSUM") as nsp,
            tc.tile_pool(name="tp", bufs=2, space="PSUM") as tpp,
            tc.tile_pool(name="gt", bufs=2) as gtp,
            tc.tile_pool(name="sc", bufs=2) as scp,
            tc.tile_pool(name="cf", bufs=2) as cfp,
            tc.tile_pool(name="pde", bufs=1) as pde,
            tc.tile_pool(name="rw", bufs=2) as rwp,
        ):
            # ============ loads & constants ============
            eye = lda.tile([N, N], f32, tag="eye")
            nc.sync.dma_start(eye[:], eye_d[:])
            sel = lda.tile([N, NL], f32, tag="sel")
            nc.sync.dma_start(sel[:], sel_d[:])
            shf = lda.tile([128, N], f32, tag="shf")
            nc.sync.dma_start(shf[:], shf_d[:])
            twoI = cst.tile([N, N], f32, tag="twoI")
            nc.vector.tensor_scalar_mul(twoI[:], eye[:], 2.0)
            ones64 = cst.tile([N, 1], f32, tag="ones64")
            nc.vector.memset(ones64[:], 1.0)
            onesrow = cst.tile([1, N], f32, tag="onesrow")
            nc.vector.memset(onesrow[:], 1.0)

            # B-side (rhs) tensors [D, (m,t)] and A-side (lhsT) [D, (n,s)]
            bt = {}
            for nm, src in (("x", xf), ("y", yf), ("z", zf)):
                t = lda.tile([D, N * S], f32, tag=f"bt_{nm}")
                nc.sync.dma_start(t[:], src.rearrange("m t d -> d (m t)"))
                bt[nm] = t
            at = {}
            for nm, src in (("x", xc), ("y", yc), ("z", zc)):
                t = lda.tile([D, NL * S], f32, tag=f"at_{nm}")
                nc.sync.dma_start(t[:], src.rearrange("n s d -> d (n s)"))
                at[nm] = t

            # ============ x2 (squared norms) for rbf ============
            xsqc = lda.tile([S, NL * D], f32, tag="xsqc")
            nc.sync.dma_start(xsqc[:].rearrange("s (n d) -> s n d", n=NL),
                              xc.rearrange("n s d -> s n d"))
            sqc = lda.tile([S, NL * D], f32, tag="sqc")
            nc.scalar.activation(sqc[:], xsqc[:], AF.Square)
            x2c = lda.tile([S, NL], f32, tag="x2c")
            nc.vector.tensor_reduce(
                x2c[:], sqc[:].rearrange("s (n d) -> s n d", n=NL),
                axis=AX.X, op=OP.add)
            nc.sync.dma_start(x2c_dr.rearrange("n s -> s n"), x2c[:])

            xsqf = lda.tile([S, N * D], f32, tag="xsqf")
            nc.sync.dma_start(xsqf[:].rearrange("t (m d) -> t m d", m=N),
                              xf.rearrange("m t d -> t m d"))
            sqf = lda.tile([S, N * D], f32, tag="sqf")
            nc.scalar.activation(sqf[:], xsqf[:], AF.Square)
            x2f = lda.tile([S, N], f32, tag="x2f")
            nc.vector.tensor_reduce(
                x2f[:], sqf[:].rearrange("t (m d) -> t m d", m=N),
                axis=AX.X, op=OP.add)
            nc.sync.dma_start(x2f_dr.rearrange("m t -> t m"), x2f[:])

            x2B = lda.tile([128, S], f32, tag="x2B")
            for h in range(2):
                nc.sync.dma_start(x2B[h * N:(h + 1) * N, :], x2f_dr[:])

            # inc storage: 16 block slots x (32x32) coarse increments
            incst = pde.tile([128, 16 * CC * CC], f32, tag="incst")

            # ============ gram pipeline ============
            def emit_gram_mms(g, a_nm, b_nm):
                """matmuls + PSUM->DRAM scatter for gram g."""
                for n in range(NL):
                    lhsT = at[a_nm][:, n * S:(n + 1) * S]
                    for (m0, mw) in MCH:
                        ps = mmp.tile([S, 15 * S], f32, tag="mm")
                        w = mw * S
                        nc.tensor.matmul(
                            ps[:, :w], lhsT,
                            bt[b_nm][:, m0 * S:(m0 + mw) * S],
                            start=True, stop=True)
                        st = scp.tile([S, 15 * S], f32, tag="mmst")
                        nc.scalar.copy(st[:, :w], ps[:, :w])
                        nc.sync.dma_start(
                            gdr[g][n, m0:m0 + mw].rearrange("m s t -> s m t"),
                            st[:, :w].rearrange("s (m t) -> s m t", t=S))

            def emit_gram_blocks(g, rbf):
                """gather to pair layout, (rbf assembly), increments."""
                for b in range(4):
                    gt = gtp.tile([128, GRID], f32, tag="gt")
                    nc.sync.dma_start(
                        gt[:],
                        gdr[g][2 * b:2 * b + 2]
                        .rearrange("h m s t -> (h m) (s t)"))
                    if rbf:
                        x2A = scp.tile([128, S], f32, tag="x2A")
                        for h in range(2):
                            nc.sync.dma_start(
                                x2A[h * N:(h + 1) * N, :],
                                x2c_dr[2 * b + h:2 * b + h + 1]
                                .broadcast_to((N, S)))
                        u = scp.tile([128, GRID], f32, tag="u")
                        nc.vector.tensor_tensor(
                            u[:].rearrange("p (s t) -> p s t", t=S),
                            x2A[:].rearrange("p (s o) -> p s o", o=1)
                            .broadcast_to((128, S, S)),
                            x2B[:].rearrange("p (o t) -> p o t", o=1)
                            .broadcast_to((128, S, S)),
                            op=OP.add)
                        v = scp.tile([128, GRID], f32, tag="v")
                        nc.vector.scalar_tensor_tensor(
                            v[:], gt[:], 2.0, u[:],
                            op0=OP.mult, op1=OP.subtract)
                        nc.scalar.activation(gt[:], v[:], AF.Exp)
                    # increments: R = G[:,1:]-G[:,:-1]; inc = R[1:,:]-R[:-1,:]
                    gv = gt[:].rearrange("p (s t) -> p s t", t=S)
                    rt = scp.tile([128, S * (S - 1)], f32, tag="rt")
                    rv = rt[:].rearrange("p (s t) -> p s t", t=S - 1)
                    nc.vector.tensor_tensor(
                        rv, gv[:, :, 1:], gv[:, :, :S - 1], op=OP.subtract)
                    slot = g * 4 + b
                    nc.vector.tensor_tensor(
                        incst[:, slot * CC * CC:(slot + 1) * CC * CC]
                        .rearrange("p (a c) -> p a c", c=CC),
                        rv[:, 1:, :], rv[:, :S - 1, :], op=OP.subtract)

            # ============ PDE row-scan solver ============
            def emit_pde(blk0, nblk, tag):
                W = nblk * 65
                c1s = pde.tile([128, W], f32, tag=f"c1s{tag}")
                dbuf = pde.tile([128, W], f32, tag=f"d{tag}")
                nc.vector.memset(c1s[:], 0.0)
                nc.vector.memset(dbuf[:], 1.0)
                prev = rwp.tile([128, W], f32, tag=f"row{tag}")
                nc.vector.memset(prev[:], 1.0)
                t1 = pde.tile([128, nblk * PQ], f32, tag=f"t1{tag}")
                t2 = pde.tile([128, nblk * PQ], f32, tag=f"t2{tag}")

                inc3 = incst[:].rearrange(
                    "p (k a c) -> p k a c", k=16, a=CC)[:, blk0:blk0 + nblk]

                for r in range(1, PQ + 1):
                    a = (r - 1) // 2
                    if r % 2 == 1:
                        # JIT coefficients for coarse row a
                        inca = inc3[:, :, a, :]            # [128, nblk, 32]
                        s12 = cfp.tile([128, nblk * CC], f32, tag=f"s12{tag}")
                        s12v = s12[:].rearrange("p (b c) -> p b c", c=CC)
                        nc.scalar.activation(s12v, inca, AF.Square,
                                             scale=SQRT192INV)
                        c2r = cfp.tile([128, nblk * CC], f32, tag=f"c2r{tag}")
                        nc.scalar.activation(c2r[:], s12[:], AF.Copy,
                                             scale=-1.0, bias=1.0)
                        vr = cfp.tile([128, nblk * CC], f32, tag=f"vr{tag}")
                        nc.scalar.activation(
                            vr[:].rearrange("p (b c) -> p b c", c=CC),
                            inca, AF.Copy, scale=0.125, bias=1.0)
                        c1r = cfp.tile([128, nblk * CC], f32, tag=f"c1r{tag}")
                        nc.vector.tensor_tensor(c1r[:], s12[:], vr[:], op=OP.add)
                        # stage expanded C1 row (x2 dyadic) into scan coeffs
                        nc.scalar.activation(
                            c1s[:].rearrange("p (b s) -> p b s", s=65)
                            [:, :, 1:65].rearrange("p b (c e) -> p b c e", e=2),
                            c1r[:].rearrange("p (b c o) -> p b c o", c=CC, o=1)
                            .broadcast_to((128, nblk, CC, 2)),
                            AF.Copy)
                        c2b = c2r[:].rearrange("p (b c o) -> p b c o", c=CC, o=1) \
                            .broadcast_to((128, nblk, CC, 2))
                        c1b = c1r[:].rearrange("p (b c o) -> p b c o", c=CC, o=1) \
                            .broadcast_to((128, nblk, CC, 2))

                    pv = prev[:].rearrange("p (b s) -> p b s", s=65)
                    t1v = t1[:].rearrange("p (b s) -> p b s", s=PQ) \
                        .rearrange("p b (c e) -> p b c e", e=2)
                    t2v = t2[:].rearrange("p (b s) -> p b s", s=PQ) \
                        .rearrange("p b (c e) -> p b c e", e=2)
                    nc.vector.tensor_tensor(
                        t1v, pv[:, :, 1:65].rearrange("p b (c e) -> p b c e", e=2),
                        c1b, op=OP.mult)
                    nc.vector.tensor_tensor(
                        t2v, pv[:, :, 0:64].rearrange("p b (c e) -> p b c e", e=2),
                        c2b, op=OP.mult)
                    nc.vector.tensor_tensor(
                        dbuf[:].rearrange("p (b s) -> p b s", s=65)[:, :, 1:65],
                        t1[:].rearrange("p (b s) -> p b s", s=PQ),
                        t2[:].rearrange("p (b s) -> p b s", s=PQ),
                        op=OP.subtract)
                    new = rwp.tile([128, W], f32, tag=f"row{tag}")
                    nc.vector.tensor_tensor_scan(
                        new[:], c1s[:], dbuf[:], 1.0,
                        op0=OP.mult, op1=OP.add)
                    prev = new
                return prev

            # ---- K gram + PDE ----
            emit_gram_mms(0, "x", "x")
            emit_gram_blocks(0, rbf=True)
            lastK = emit_pde(0, 4, "K")
            kvals = cst.tile([128, 4], f32, tag="kvals")
            nc.vector.tensor_copy(
                kvals[:].rearrange("p (b o) -> p b o", o=1),
                lastK[:].rearrange("p (b s) -> p b s", s=65)[:, :, 64:65])
            nc.sync.dma_start(ksh_dr.rearrange("b p -> p b"), kvals[:])
            nc.gpsimd.collective_compute(
                "AllGather", mybir.AluOpType.bypass,
                replica_groups=[list(range(N_CORES))],
                ins=[ksh_dr[:]], outs=[kall_dr[:]])
            kt = cst.tile([N, N], f32, tag="kt")
            nc.sync.dma_start(kt[:], kall_dr.rearrange("c b h m -> (c b h) m"))

            # ---- L grams (overlap with K PDE on other engines) ----
            emit_gram_mms(1, "y", "y")
            emit_gram_mms(2, "z", "z")
            emit_gram_mms(3, "y", "z")
            emit_gram_blocks(1, rbf=False)
            emit_gram_blocks(2, rbf=False)
            emit_gram_blocks(3, rbf=False)

            # ---- Newton-Schulz inverse of M = K + I (replicated) ----
            mt = cst.tile([N, N], f32, tag="mt")
            nc.vector.tensor_tensor(mt[:], kt[:], eye[:], op=OP.add)
            r64 = cst.tile([N, 1], f32, tag="r64")
            nc.vector.tensor_reduce(r64[:], mt[:], axis=AX.X, op=OP.add)
            rT = tpp.tile([1, N], f32, tag="tp")
            nc.tensor.transpose(rT[:], r64[:], eye[:])
            rmax = cst.tile([1, 1], f32, tag="rmax")
            nc.vector.tensor_reduce(rmax[:], rT[:], axis=AX.X, op=OP.max)
            alpha = cst.tile([1, 1], f32, tag="alpha")
            nc.vector.reciprocal(alpha[:], rmax[:])
            alps = tpp.tile([N, 1], f32, tag="tp")
            nc.tensor.matmul(alps[:], onesrow[:], alpha[:], start=True, stop=True)
            alpb = cst.tile([N, 1], f32, tag="alpb")
            nc.scalar.copy(alpb[:], alps[:])
            xns = cst.tile([N, N], f32, tag="xns")
            nc.vector.tensor_scalar_mul(xns[:], eye[:], alpb[:])
            tt = cst.tile([N, N], f32, tag="tt")
            for _ in range(NS_ITERS):
                p1 = nsp.tile([N, N], f32, tag="ns")
                nc.tensor.matmul(p1[:], mt[:], xns[:], start=True, stop=True)
                nc.vector.scalar_tensor_tensor(
                    tt[:], p1[:], -1.0, twoI[:], op0=OP.mult, op1=OP.add)
                p2 = nsp.tile([N, N], f32, tag="ns")
                nc.tensor.matmul(p2[:], xns[:], tt[:], start=True, stop=True)
                nc.scalar.copy(xns[:], p2[:])
            # E = B - B^2
            p3 = nsp.tile([N, N], f32, tag="ns")
            nc.tensor.matmul(p3[:], xns[:], xns[:], start=True, stop=True)
            # et = xns - p3 : (p3 * -1) + xns
            et = cst.tile([N, N], f32, tag="et")
            nc.vector.scalar_tensor_tensor(
                et[:], p3[:], -1.0, xns[:], op0=OP.mult, op1=OP.add)
            ecp = nsp.tile([N, NL], f32, tag="ns")
            nc.tensor.matmul(ecp[:], et[:], sel[:], start=True, stop=True)
            ecols = cst.tile([N, NL], f32, tag="ecols")
            nc.scalar.copy(ecols[:], ecp[:])

            # ---- L PDE + partial trace ----
            lastL = emit_pde(4, 12, "L")
            lvals = cst.tile([128, 12], f32, tag="lvals")
            nc.vector.tensor_copy(
                lvals[:].rearrange("p (b o) -> p b o", o=1),
                lastL[:].rearrange("p (b s) -> p b s", s=65)[:, :, 64:65])
            lsum = cst.tile([128, 4], f32, tag="lsum")
            nc.vector.tensor_tensor(lsum[:], lvals[:, 0:4], lvals[:, 4:8],
                                    op=OP.add)
            nc.vector.scalar_tensor_tensor(
                lsum[:], lvals[:, 8:12], -2.0, lsum[:], op0=OP.mult, op1=OP.add)
            lup_p = tpp.tile([N, 4], f32, tag="tp")
            nc.tensor.matmul(lup_p[:], shf[:], lsum[:], start=True, stop=True)
            lup = cst.tile([N, 4], f32, tag="lup")
            nc.scalar.copy(lup[:], lup_p[:])
            prodA = cst.tile([N, 4], f32, tag="prodA")
            nc.vector.tensor_tensor(
                prodA[:], lsum[0:N, :],
                ecols[:].rearrange("p (c e) -> p c e", e=2)[:, :, 0],
                op=OP.mult)
            prodB = cst.tile([N, 4], f32, tag="prodB")
            nc.vector.tensor_tensor(
                prodB[:], lup[:],
                ecols[:].rearrange("p (c e) -> p c e", e=2)[:, :, 1],
                op=OP.mult)
            ra = cst.tile([N, 1], f32, tag="ra")
            nc.vector.tensor_reduce(ra[:], prodA[:], axis=AX.X, op=OP.add)
            rb = cst.tile([N, 1], f32, tag="rb")
            nc.vector.tensor_reduce(rb[:], prodB[:], axis=AX.X, op=OP.add)
            vsum = cst.tile([N, 1], f32, tag="vsum")
            nc.vector.tensor_tensor(vsum[:], ra[:], rb[:], op=OP.add)
            part = tpp.tile([1, 1], f32, tag="tp")
            nc.tensor.matmul(part[:], vsum[:], ones64[:], start=True, stop=True)
            outst = cst.tile([1, 1], f32, tag="outst")
            nc.scalar.copy(outst[:], part[:])
            nc.sync.dma_start(out_d[:], outst[:])

    nc.compile()
    return nc


def _host_inputs(x, y, z):
    eye = np.eye(N, dtype=np.float32)
    shf = np.zeros((128, N), dtype=np.float32)
    for p in range(N):
        shf[p + N, p] = 1.0
    maps = []
    for c in range(N_CORES):
        sel = np.zeros((N, NL), dtype=np.float32)
        for j in range(NL):
            sel[NL * c + j, j] = 1.0
        maps.append({
            "xf": np.ascontiguousarray(x), "yf": np.ascontiguousarray(y),
            "zf": np.ascontiguousarray(z),
            "xc": np.ascontiguousarray(x[NL * c:NL * (c + 1)]),
            "yc": np.ascontiguousarray(y[NL * c:NL * (c + 1)]),
            "zc": np.ascontiguousarray(z[NL * c:NL * (c + 1)]),
            "eye": eye, "sel": sel, "shf": shf,
        })
    return maps


def kernel(x, y, z):
    from concourse import bass_utils
    if "nc" not in _CACHE:
        _CACHE["nc"] = _build()
    nc = _CACHE["nc"]
    maps = _host_inputs(np.asarray(x, np.float32), np.asarray(y, np.float32),
                        np.asarray(z, np.float32))
    res = bass_utils.run_bass_kernel_spmd(nc, maps, core_ids=list(range(N_CORES)))
    total = np.float64(0.0)
    for c in range(N_CORES):
        total += np.float64(res.results[c]["out"][0, 0])
    return np.float32(total)



# revision 3
# speedup vs baseline: 4.0887x; 4.0887x over previous
"""ConditionalSigKerMMD discriminator loss on 8 TRN2 NeuronCores.

Strategy (self-contained, hardcoded for x,y,z of shape (64,33,8) fp32):
  - 4 signature-kernel Grams (K=rbf(x,x), Lgen=lin(y,y), Ltrue=lin(z,z),
    Lmix=lin(y,z)), each 64x64 pairs. Pair dim sharded 8 ways by row
    block (8 rows/core -> 512 pairs/gram/core, 2048 pairs/core total).
  - Static Gram via PE matmul in (n,s)x(m,t) layout, bounced through DRAM
    into pair-major layout [pair, (s,t)].
  - Goursat PDE solved as 64 row recurrences using the DVE
    tensor_tensor_scan: state = C1*state + D, with per-block reset slots.
  - K shards AllGathered; (K+I)^-1 via Newton-Schulz (replicated);
    E = B - B^2; partial trace tr(E*Lsum) per core; host sums 8 scalars.

Execution path: the NEFF-wrapped executable and the constant inputs are
built once and cached; each call uploads one packed (3,64,33,8) buffer,
replicates/slices it on-device in a small prep jit, runs the Bass NEFF
on all 8 cores, and blocks on a single fetch of the 8 partial sums.
This keeps each call to ~one tunnel round-trip (the baseline rebuilt the
jit per call, paying re-trace + re-lower + extra round trips).
"""
import numpy as np

N_CORES = 8
N = 64           # pairs per gram row/col
S = 33           # path length
D = 8            # path dim
NL = 8           # rows (n values) per core
PQ = 64          # fine grid size (S-1)*2
CC = 32          # coarse coefficient grid
GRID = S * S     # 1089
SQRT192INV = 1.0 / np.sqrt(192.0)
NS_ITERS = 14

_CACHE = {}


def _build():
    import concourse.bass as bass
    import concourse.mybir as mybir
    import concourse.tile as tile
    import concourse.bacc as bacc

    f32 = mybir.dt.float32
    AX = mybir.AxisListType
    OP = mybir.AluOpType
    AF = mybir.ActivationFunctionType

    nc = bacc.Bacc("TRN2", target_bir_lowering=False, debug=False,
                   num_devices=N_CORES)

    # ---- I/O ----
    xf = nc.dram_tensor("xf", [N, S, D], f32, kind="ExternalInput").ap()
    yf = nc.dram_tensor("yf", [N, S, D], f32, kind="ExternalInput").ap()
    zf = nc.dram_tensor("zf", [N, S, D], f32, kind="ExternalInput").ap()
    xc = nc.dram_tensor("xc", [NL, S, D], f32, kind="ExternalInput").ap()
    yc = nc.dram_tensor("yc", [NL, S, D], f32, kind="ExternalInput").ap()
    zc = nc.dram_tensor("zc", [NL, S, D], f32, kind="ExternalInput").ap()
    eye_d = nc.dram_tensor("eye", [N, N], f32, kind="ExternalInput").ap()
    sel_d = nc.dram_tensor("sel", [N, NL], f32, kind="ExternalInput").ap()
    shf_d = nc.dram_tensor("shf", [128, N], f32, kind="ExternalInput").ap()
    out_d = nc.dram_tensor("out", [1, 1], f32, kind="ExternalOutput").ap()

    # ---- internal DRAM ----
    gdr = [nc.dram_tensor(f"gram{g}", [NL, N, S, S], f32).ap() for g in range(4)]
    x2c_dr = nc.dram_tensor("x2c", [NL, S], f32).ap()
    x2f_dr = nc.dram_tensor("x2f", [N, S], f32).ap()
    ksh_dr = nc.dram_tensor("ksh", [4, 128], f32).ap()
    kall_dr = nc.dram_tensor("kall", [N_CORES, 4, 2, N], f32,
                             addr_space="Shared").ap()

    # matmul m-chunks (PSUM bank <=512 fp32, m-aligned for scatter APs)
    MCH = [(0, 15), (15, 15), (30, 15), (45, 15), (60, 4)]

    with tile.TileContext(nc) as tc:
        with (
            tc.tile_pool(name="cst", bufs=1) as cst,
            tc.tile_pool(name="lda", bufs=1) as lda,
            tc.tile_pool(name="mmp", bufs=4, space="PSUM") as mmp,
            tc.tile_pool(name="nsp", bufs=2, space="PSUM") as nsp,
            tc.tile_pool(name="tp", bufs=2, space="PSUM") as tpp,
            tc.tile_pool(name="gt", bufs=2) as gtp,
            tc.tile_pool(name="sc", bufs=2) as scp,
            tc.tile_pool(name="cf", bufs=2) as cfp,
            tc.tile_pool(name="pde", bufs=1) as pde,
            tc.tile_pool(name="rw", bufs=2) as rwp,
        ):
            # ============ loads & constants ============
            eye = lda.tile([N, N], f32, tag="eye")
            nc.sync.dma_start(eye[:], eye_d[:])
            sel = lda.tile([N, NL], f32, tag="sel")
            nc.sync.dma_start(sel[:], sel_d[:])
            shf = lda.tile([128, N], f32, tag="shf")
            nc.sync.dma_start(shf[:], shf_d[:])
            twoI = cst.tile([N, N], f32, tag="twoI")
            nc.vector.tensor_scalar_mul(twoI[:], eye[:], 2.0)
            ones64 = cst.tile([N, 1], f32, tag="ones64")
            nc.vector.memset(ones64[:], 1.0)
            onesrow = cst.tile([1, N], f32, tag="onesrow")
            nc.vector.memset(onesrow[:], 1.0)

            # B-side (rhs) tensors [D, (m,t)] and A-side (lhsT) [D, (n,s)]
            bt = {}
            for nm, src in (("x", xf), ("y", yf), ("z", zf)):
                t = lda.tile([D, N * S], f32, tag=f"bt_{nm}")
                nc.sync.dma_start(t[:], src.rearrange("m t d -> d (m t)"))
                bt[nm] = t
            at = {}
            for nm, src in (("x", xc), ("y", yc), ("z", zc)):
                t = lda.tile([D, NL * S], f32, tag=f"at_{nm}")
                nc.sync.dma_start(t[:], src.rearrange("n s d -> d (n s)"))
                at[nm] = t

            # ============ x2 (squared norms) for rbf ============
            xsqc = lda.tile([S, NL * D], f32, tag="xsqc")
            nc.sync.dma_start(xsqc[:].rearrange("s (n d) -> s n d", n=NL),
                              xc.rearrange("n s d -> s n d"))
            sqc = lda.tile([S, NL * D], f32, tag="sqc")
            nc.scalar.activation(sqc[:], xsqc[:], AF.Square)
            x2c = lda.tile([S, NL], f32, tag="x2c")
            nc.vector.tensor_reduce(
                x2c[:], sqc[:].rearrange("s (n d) -> s n d", n=NL),
                axis=AX.X, op=OP.add)
            nc.sync.dma_start(x2c_dr.rearrange("n s -> s n"), x2c[:])

            xsqf = lda.tile([S, N * D], f32, tag="xsqf")
            nc.sync.dma_start(xsqf[:].rearrange("t (m d) -> t m d", m=N),
                              xf.rearrange("m t d -> t m d"))
            sqf = lda.tile([S, N * D], f32, tag="sqf")
            nc.scalar.activation(sqf[:], xsqf[:], AF.Square)
            x2f = lda.tile([S, N], f32, tag="x2f")
            nc.vector.tensor_reduce(
                x2f[:], sqf[:].rearrange("t (m d) -> t m d", m=N),
                axis=AX.X, op=OP.add)
            nc.sync.dma_start(x2f_dr.rearrange("m t -> t m"), x2f[:])

            x2B = lda.tile([128, S], f32, tag="x2B")
            for h in range(2):
                nc.sync.dma_start(x2B[h * N:(h + 1) * N, :], x2f_dr[:])

            # inc storage: 16 block slots x (32x32) coarse increments
            incst = pde.tile([128, 16 * CC * CC], f32, tag="incst")

            # ============ gram pipeline ============
            def emit_gram_mms(g, a_nm, b_nm):
                """matmuls + PSUM->DRAM scatter for gram g."""
                for n in range(NL):
                    lhsT = at[a_nm][:, n * S:(n + 1) * S]
                    for (m0, mw) in MCH:
                        ps = mmp.tile([S, 15 * S], f32, tag="mm")
                        w = mw * S
                        nc.tensor.matmul(
                            ps[:, :w], lhsT,
                            bt[b_nm][:, m0 * S:(m0 + mw) * S],
                            start=True, stop=True)
                        st = scp.tile([S, 15 * S], f32, tag="mmst")
                        nc.scalar.copy(st[:, :w], ps[:, :w])
                        nc.sync.dma_start(
                            gdr[g][n, m0:m0 + mw].rearrange("m s t -> s m t"),
                            st[:, :w].rearrange("s (m t) -> s m t", t=S))

            def emit_gram_blocks(g, rbf):
                """gather to pair layout, (rbf assembly), increments."""
                for b in range(4):
                    gt = gtp.tile([128, GRID], f32, tag="gt")
                    nc.sync.dma_start(
                        gt[:],
                        gdr[g][2 * b:2 * b + 2]
                        .rearrange("h m s t -> (h m) (s t)"))
                    if rbf:
                        x2A = scp.tile([128, S], f32, tag="x2A")
                        for h in range(2):
                            nc.sync.dma_start(
                                x2A[h * N:(h + 1) * N, :],
                                x2c_dr[2 * b + h:2 * b + h + 1]
                                .broadcast_to((N, S)))
                        u = scp.tile([128, GRID], f32, tag="u")
                        nc.vector.tensor_tensor(
                            u[:].rearrange("p (s t) -> p s t", t=S),
                            x2A[:].rearrange("p (s o) -> p s o", o=1)
                            .broadcast_to((128, S, S)),
                            x2B[:].rearrange("p (o t) -> p o t", o=1)
                            .broadcast_to((128, S, S)),
                            op=OP.add)
                        v = scp.tile([128, GRID], f32, tag="v")
                        nc.vector.scalar_tensor_tensor(
                            v[:], gt[:], 2.0, u[:],
                            op0=OP.mult, op1=OP.subtract)
                        nc.scalar.activation(gt[:], v[:], AF.Exp)
                    # increments: R = G[:,1:]-G[:,:-1]; inc = R[1:,:]-R[:-1,:]
                    gv = gt[:].rearrange("p (s t) -> p s t", t=S)
                    rt = scp.tile([128, S * (S - 1)], f32, tag="rt")
                    rv = rt[:].rearrange("p (s t) -> p s t", t=S - 1)
                    nc.vector.tensor_tensor(
                        rv, gv[:, :, 1:], gv[:, :, :S - 1], op=OP.subtract)
                    slot = g * 4 + b
                    nc.vector.tensor_tensor(
                        incst[:, slot * CC * CC:(slot + 1) * CC * CC]
                        .rearrange("p (a c) -> p a c", c=CC),
                        rv[:, 1:, :], rv[:, :S - 1, :], op=OP.subtract)

            # ============ PDE row-scan solver ============
            def emit_pde(blk0, nblk, tag):
                W = nblk * 65
                c1s = pde.tile([128, W], f32, tag=f"c1s{tag}")
                dbuf = pde.tile([128, W], f32, tag=f"d{tag}")
                nc.vector.memset(c1s[:], 0.0)
                nc.vector.memset(dbuf[:], 1.0)
                prev = rwp.tile([128, W], f32, tag=f"row{tag}")
                nc.vector.memset(prev[:], 1.0)
                t1 = pde.tile([128, nblk * PQ], f32, tag=f"t1{tag}")
                t2 = pde.tile([128, nblk * PQ], f32, tag=f"t2{tag}")

                inc3 = incst[:].rearrange(
                    "p (k a c) -> p k a c", k=16, a=CC)[:, blk0:blk0 + nblk]

                for r in range(1, PQ + 1):
                    a = (r - 1) // 2
                    if r % 2 == 1:
                        # JIT coefficients for coarse row a
                        inca = inc3[:, :, a, :]            # [128, nblk, 32]
                        s12 = cfp.tile([128, nblk * CC], f32, tag=f"s12{tag}")
                        s12v = s12[:].rearrange("p (b c) -> p b c", c=CC)
                        nc.scalar.activation(s12v, inca, AF.Square,
                                             scale=SQRT192INV)
                        c2r = cfp.tile([128, nblk * CC], f32, tag=f"c2r{tag}")
                        nc.scalar.activation(c2r[:], s12[:], AF.Copy,
                                             scale=-1.0, bias=1.0)
                        vr = cfp.tile([128, nblk * CC], f32, tag=f"vr{tag}")
                        nc.scalar.activation(
                            vr[:].rearrange("p (b c) -> p b c", c=CC),
                            inca, AF.Copy, scale=0.125, bias=1.0)
                        c1r = cfp.tile([128, nblk * CC], f32, tag=f"c1r{tag}")
                        nc.vector.tensor_tensor(c1r[:], s12[:], vr[:], op=OP.add)
                        # stage expanded C1 row (x2 dyadic) into scan coeffs
                        nc.scalar.activation(
                            c1s[:].rearrange("p (b s) -> p b s", s=65)
                            [:, :, 1:65].rearrange("p b (c e) -> p b c e", e=2),
                            c1r[:].rearrange("p (b c o) -> p b c o", c=CC, o=1)
                            .broadcast_to((128, nblk, CC, 2)),
                            AF.Copy)
                        c2b = c2r[:].rearrange("p (b c o) -> p b c o", c=CC, o=1) \
                            .broadcast_to((128, nblk, CC, 2))
                        c1b = c1r[:].rearrange("p (b c o) -> p b c o", c=CC, o=1) \
                            .broadcast_to((128, nblk, CC, 2))

                    pv = prev[:].rearrange("p (b s) -> p b s", s=65)
                    t1v = t1[:].rearrange("p (b s) -> p b s", s=PQ) \
                        .rearrange("p b (c e) -> p b c e", e=2)
                    t2v = t2[:].rearrange("p (b s) -> p b s", s=PQ) \
                        .rearrange("p b (c e) -> p b c e", e=2)
                    nc.vector.tensor_tensor(
                        t1v, pv[:, :, 1:65].rearrange("p b (c e) -> p b c e", e=2),
                        c1b, op=OP.mult)
                    nc.vector.tensor_tensor(
                        t2v, pv[:, :, 0:64].rearrange("p b (c e) -> p b c e", e=2),
                        c2b, op=OP.mult)
                    nc.vector.tensor_tensor(
                        dbuf[:].rearrange("p (b s) -> p b s", s=65)[:, :, 1:65],
                        t1[:].rearrange("p (b s) -> p b s", s=PQ),
                        t2[:].rearrange("p (b s) -> p b s", s=PQ),
                        op=OP.subtract)
                    new = rwp.tile([128, W], f32, tag=f"row{tag}")
                    nc.vector.tensor_tensor_scan(
                        new[:], c1s[:], dbuf[:], 1.0,
                        op0=OP.mult, op1=OP.add)
                    prev = new
                return prev

            # ---- K gram + PDE ----
            emit_gram_mms(0, "x", "x")
            emit_gram_blocks(0, rbf=True)
            lastK = emit_pde(0, 4, "K")
            kvals = cst.tile([128, 4], f32, tag="kvals")
            nc.vector.tensor_copy(
                kvals[:].rearrange("p (b o) -> p b o", o=1),
                lastK[:].rearrange("p (b s) -> p b s", s=65)[:, :, 64:65])
            nc.sync.dma_start(ksh_dr.rearrange("b p -> p b"), kvals[:])
            nc.gpsimd.collective_compute(
                "AllGather", mybir.AluOpType.bypass,
                replica_groups=[list(range(N_CORES))],
                ins=[ksh_dr[:]], outs=[kall_dr[:]])
            kt = cst.tile([N, N], f32, tag="kt")
            nc.sync.dma_start(kt[:], kall_dr.rearrange("c b h m -> (c b h) m"))

            # ---- L grams (overlap with K PDE on other engines) ----
            emit_gram_mms(1, "y", "y")
            emit_gram_mms(2, "z", "z")
            emit_gram_mms(3, "y", "z")
            emit_gram_blocks(1, rbf=False)
            emit_gram_blocks(2, rbf=False)
            emit_gram_blocks(3, rbf=False)

            # ---- Newton-Schulz inverse of M = K + I (replicated) ----
            mt = cst.tile([N, N], f32, tag="mt")
            nc.vector.tensor_tensor(mt[:], kt[:], eye[:], op=OP.add)
            r64 = cst.tile([N, 1], f32, tag="r64")
            nc.vector.tensor_reduce(r64[:], mt[:], axis=AX.X, op=OP.add)
            rT = tpp.tile([1, N], f32, tag="tp")
            nc.tensor.transpose(rT[:], r64[:], eye[:])
            rmax = cst.tile([1, 1], f32, tag="rmax")
            nc.vector.tensor_reduce(rmax[:], rT[:], axis=AX.X, op=OP.max)
            alpha = cst.tile([1, 1], f32, tag="alpha")
            nc.vector.reciprocal(alpha[:], rmax[:])
            alps = tpp.tile([N, 1], f32, tag="tp")
            nc.tensor.matmul(alps[:], onesrow[:], alpha[:], start=True, stop=True)
            alpb = cst.tile([N, 1], f32, tag="alpb")
            nc.scalar.copy(alpb[:], alps[:])
            xns = cst.tile([N, N], f32, tag="xns")
            nc.vector.tensor_scalar_mul(xns[:], eye[:], alpb[:])
            tt = cst.tile([N, N], f32, tag="tt")
            for _ in range(NS_ITERS):
                p1 = nsp.tile([N, N], f32, tag="ns")
                nc.tensor.matmul(p1[:], mt[:], xns[:], start=True, stop=True)
                nc.vector.scalar_tensor_tensor(
                    tt[:], p1[:], -1.0, twoI[:], op0=OP.mult, op1=OP.add)
                p2 = nsp.tile([N, N], f32, tag="ns")
                nc.tensor.matmul(p2[:], xns[:], tt[:], start=True, stop=True)
                nc.scalar.copy(xns[:], p2[:])
            # E = B - B^2
            p3 = nsp.tile([N, N], f32, tag="ns")
            nc.tensor.matmul(p3[:], xns[:], xns[:], start=True, stop=True)
            # et = xns - p3 : (p3 * -1) + xns
            et = cst.tile([N, N], f32, tag="et")
            nc.vector.scalar_tensor_tensor(
                et[:], p3[:], -1.0, xns[:], op0=OP.mult, op1=OP.add)
            ecp = nsp.tile([N, NL], f32, tag="ns")
            nc.tensor.matmul(ecp[:], et[:], sel[:], start=True, stop=True)
            ecols = cst.tile([N, NL], f32, tag="ecols")
            nc.scalar.copy(ecols[:], ecp[:])

            # ---- L PDE + partial trace ----
            lastL = emit_pde(4, 12, "L")
            lvals = cst.tile([128, 12], f32, tag="lvals")
            nc.vector.tensor_copy(
                lvals[:].rearrange("p (b o) -> p b o", o=1),
                lastL[:].rearrange("p (b s) -> p b s", s=65)[:, :, 64:65])
            lsum = cst.tile([128, 4], f32, tag="lsum")
            nc.vector.tensor_tensor(lsum[:], lvals[:, 0:4], lvals[:, 4:8],
                                    op=OP.add)
            nc.vector.scalar_tensor_tensor(
                lsum[:], lvals[:, 8:12], -2.0, lsum[:], op0=OP.mult, op1=OP.add)
            lup_p = tpp.tile([N, 4], f32, tag="tp")
            nc.tensor.matmul(lup_p[:], shf[:], lsum[:], start=True, stop=True)
            lup = cst.tile([N, 4], f32, tag="lup")
            nc.scalar.copy(lup[:], lup_p[:])
            prodA = cst.tile([N, 4], f32, tag="prodA")
            nc.vector.tensor_tensor(
                prodA[:], lsum[0:N, :],
                ecols[:].rearrange("p (c e) -> p c e", e=2)[:, :, 0],
                op=OP.mult)
            prodB = cst.tile([N, 4], f32, tag="prodB")
            nc.vector.tensor_tensor(
                prodB[:], lup[:],
                ecols[:].rearrange("p (c e) -> p c e", e=2)[:, :, 1],
                op=OP.mult)
            ra = cst.tile([N, 1], f32, tag="ra")
            nc.vector.tensor_reduce(ra[:], prodA[:], axis=AX.X, op=OP.add)
            rb = cst.tile([N, 1], f32, tag="rb")
            nc.vector.tensor_reduce(rb[:], prodB[:], axis=AX.X, op=OP.add)
            vsum = cst.tile([N, 1], f32, tag="vsum")
            nc.vector.tensor_tensor(vsum[:], ra[:], rb[:], op=OP.add)
            part = tpp.tile([1, 1], f32, tag="tp")
            nc.tensor.matmul(part[:], vsum[:], ones64[:], start=True, stop=True)
            outst = cst.tile([1, 1], f32, tag="outst")
            nc.scalar.copy(outst[:], part[:])
            nc.sync.dma_start(out_d[:], outst[:])

    nc.compile()
    return nc


def _host_inputs(x, y, z):
    eye = np.eye(N, dtype=np.float32)
    shf = np.zeros((128, N), dtype=np.float32)
    for p in range(N):
        shf[p + N, p] = 1.0
    maps = []
    for c in range(N_CORES):
        sel = np.zeros((N, NL), dtype=np.float32)
        for j in range(NL):
            sel[NL * c + j, j] = 1.0
        maps.append({
            "xf": np.ascontiguousarray(x), "yf": np.ascontiguousarray(y),
            "zf": np.ascontiguousarray(z),
            "xc": np.ascontiguousarray(x[NL * c:NL * (c + 1)]),
            "yc": np.ascontiguousarray(y[NL * c:NL * (c + 1)]),
            "zc": np.ascontiguousarray(z[NL * c:NL * (c + 1)]),
            "eye": eye, "sel": sel, "shf": shf,
        })
    return maps


def _make_runtime():
    """Build the Bass module once and wrap it in a cached jitted callable.

    Per call only x,y,z move host->device (one packed buffer); the
    replication to per-core full copies, the per-core row slices, and the
    donated zero output buffer are all produced on-device by a prep jit.
    Constant inputs (eye/sel/shf) stay resident across calls.
    """
    import jax
    import jax.numpy as jnp
    from jax.experimental.shard_map import shard_map
    from jax.sharding import Mesh, NamedSharding, PartitionSpec
    from concourse import bass2jax, mybir

    nc = _build()
    bass2jax.install_neuronx_cc_hook()

    partition_name = nc.partition_id_tensor.name if nc.partition_id_tensor else None
    in_names, out_names, out_avals = [], [], []
    for alloc in nc.m.functions[0].allocations:
        if not isinstance(alloc, mybir.MemoryLocationSet):
            continue
        name = alloc.memorylocations[0].name
        if alloc.kind == "ExternalInput":
            if name != partition_name:
                in_names.append(name)
        elif alloc.kind == "ExternalOutput":
            out_names.append(name)
            out_avals.append(jax.core.ShapedArray(
                tuple(alloc.tensor_shape), mybir.dt.np(alloc.dtype)))
    n_params = len(in_names)
    in_names_full = in_names + out_names + (
        [partition_name] if partition_name else [])
    donate = tuple(range(n_params, n_params + len(out_names)))

    def _body(*args):
        operands = list(args)
        if partition_name is not None:
            operands.append(bass2jax.partition_id_tensor())
        return tuple(bass2jax._bass_exec_p.bind(
            *operands, out_avals=tuple(out_avals),
            in_names=tuple(in_names_full), out_names=tuple(out_names),
            lowering_input_output_aliases=(),
            sim_require_finite=True, sim_require_nnan=True, nc=nc))

    devices = jax.devices()[:N_CORES]
    mesh = Mesh(np.asarray(devices), ("core",))
    shard = NamedSharding(mesh, PartitionSpec("core"))
    repl = NamedSharding(mesh, PartitionSpec())
    in_specs = (PartitionSpec("core"),) * (n_params + len(out_names))
    out_specs = (PartitionSpec("core"),) * len(out_names)
    sharded = jax.jit(
        shard_map(_body, mesh=mesh, in_specs=in_specs, out_specs=out_specs,
                  check_rep=False),
        donate_argnums=donate, keep_unused=True)

    # constants, staged device-resident once (global = per-core concat)
    eye = np.eye(N, dtype=np.float32)
    shf = np.zeros((128, N), dtype=np.float32)
    for p in range(N):
        shf[p + N, p] = 1.0
    sel_g = np.zeros((N_CORES, N, NL), np.float32)
    for c in range(N_CORES):
        for j in range(NL):
            sel_g[c, NL * c + j, j] = 1.0
    const = {
        "eye": jax.device_put(np.tile(eye, (N_CORES, 1)), shard),
        "sel": jax.device_put(sel_g.reshape(N_CORES * N, NL), shard),
        "shf": jax.device_put(np.tile(shf, (N_CORES, 1)), shard),
    }
    jax.block_until_ready(list(const.values()))

    def _prep(packed):
        x, y, z = packed[0], packed[1], packed[2]
        xf = jnp.tile(x, (N_CORES, 1, 1))
        yf = jnp.tile(y, (N_CORES, 1, 1))
        zf = jnp.tile(z, (N_CORES, 1, 1))
        zer = jnp.zeros((N_CORES, 1, 1), jnp.float32)
        return xf, yf, zf, x, y, z, zer

    prep = jax.jit(_prep, in_shardings=(repl,), out_shardings=(shard,) * 7)
    name2idx = {nm: i for i, nm in enumerate(in_names)}

    def call(x, y, z):
        packed = np.stack([x, y, z])
        xf, yf, zf, xc, yc, zc, zer = prep(packed)
        args = [None] * n_params
        args[name2idx["xf"]] = xf
        args[name2idx["yf"]] = yf
        args[name2idx["zf"]] = zf
        args[name2idx["xc"]] = xc
        args[name2idx["yc"]] = yc
        args[name2idx["zc"]] = zc
        for nm, buf in const.items():
            args[name2idx[nm]] = buf
        out = sharded(*args, zer)
        vals = np.asarray(out[0]).reshape(N_CORES)
        return np.float32(np.float64(vals).sum())

    return call


def _kernel_fallback(x, y, z):
    from concourse import bass_utils
    if "nc" not in _CACHE:
        _CACHE["nc"] = _build()
    nc = _CACHE["nc"]
    maps = _host_inputs(x, y, z)
    res = bass_utils.run_bass_kernel_spmd(nc, maps, core_ids=list(range(N_CORES)))
    total = np.float64(0.0)
    for c in range(N_CORES):
        total += np.float64(res.results[c]["out"][0, 0])
    return np.float32(total)


def kernel(x, y, z):
    x = np.ascontiguousarray(np.asarray(x, np.float32))
    y = np.ascontiguousarray(np.asarray(y, np.float32))
    z = np.ascontiguousarray(np.asarray(z, np.float32))
    if _CACHE.get("rt_failed"):
        return _kernel_fallback(x, y, z)
    try:
        if "rt" not in _CACHE:
            _CACHE["rt"] = _make_runtime()
        return _CACHE["rt"](x, y, z)
    except Exception:
        _CACHE["rt_failed"] = True
        return _kernel_fallback(x, y, z)



# revision 4
# speedup vs baseline: 4.2774x; 1.0461x over previous
"""ConditionalSigKerMMD discriminator loss on 8 TRN2 NeuronCores.

Strategy (self-contained, hardcoded for x,y,z of shape (64,33,8) fp32):
  - 4 signature-kernel Grams (K=rbf(x,x), Lgen=lin(y,y), Ltrue=lin(z,z),
    Lmix=lin(y,z)), each 64x64 pairs. Pair dim sharded 8 ways by row
    block (8 rows/core -> 512 pairs/gram/core, 2048 pairs/core total).
  - Static Gram via PE matmul in (n,s)x(m,t) layout, bounced through DRAM
    into pair-major layout [pair, (s,t)].
  - Goursat PDE solved as 64 row recurrences using the DVE
    tensor_tensor_scan: state = C1*state + D, with per-block reset slots.
  - K shards AllGathered; (K+I)^-1 via Newton-Schulz (replicated);
    E = B - B^2; partial trace tr(E*Lsum) per core; host sums 8 scalars.

Execution path: the NEFF-wrapped executable and the constant inputs are
built once and cached; each call uploads one packed (3,64,33,8) buffer,
replicates/slices it on-device in a small prep jit, runs the Bass NEFF
on all 8 cores, and blocks on a single fetch of the 8 partial sums.
This keeps each call to ~one tunnel round-trip (the baseline rebuilt the
jit per call, paying re-trace + re-lower + extra round trips).
"""
import numpy as np

N_CORES = 8
N = 64           # pairs per gram row/col
S = 33           # path length
D = 8            # path dim
NL = 8           # rows (n values) per core
PQ = 64          # fine grid size (S-1)*2
CC = 32          # coarse coefficient grid
GRID = S * S     # 1089
SQRT192INV = 1.0 / np.sqrt(192.0)
NS_ITERS = 14

_CACHE = {}


def _build():
    import concourse.bass as bass
    import concourse.mybir as mybir
    import concourse.tile as tile
    import concourse.bacc as bacc

    f32 = mybir.dt.float32
    AX = mybir.AxisListType
    OP = mybir.AluOpType
    AF = mybir.ActivationFunctionType

    nc = bacc.Bacc("TRN2", target_bir_lowering=False, debug=False,
                   num_devices=N_CORES)

    # ---- I/O ----
    xf = nc.dram_tensor("xf", [N, S, D], f32, kind="ExternalInput").ap()
    yf = nc.dram_tensor("yf", [N, S, D], f32, kind="ExternalInput").ap()
    zf = nc.dram_tensor("zf", [N, S, D], f32, kind="ExternalInput").ap()
    xc = nc.dram_tensor("xc", [NL, S, D], f32, kind="ExternalInput").ap()
    yc = nc.dram_tensor("yc", [NL, S, D], f32, kind="ExternalInput").ap()
    zc = nc.dram_tensor("zc", [NL, S, D], f32, kind="ExternalInput").ap()
    eye_d = nc.dram_tensor("eye", [N, N], f32, kind="ExternalInput").ap()
    sel_d = nc.dram_tensor("sel", [N, NL], f32, kind="ExternalInput").ap()
    shf_d = nc.dram_tensor("shf", [128, N], f32, kind="ExternalInput").ap()
    out_d = nc.dram_tensor("out", [1, 1], f32, kind="ExternalOutput").ap()

    # ---- internal DRAM ----
    gdr = [nc.dram_tensor(f"gram{g}", [NL, N, S, S], f32).ap() for g in range(4)]
    x2c_dr = nc.dram_tensor("x2c", [NL, S], f32).ap()
    x2f_dr = nc.dram_tensor("x2f", [N, S], f32).ap()
    ksh_dr = nc.dram_tensor("ksh", [4, 128], f32).ap()
    kall_dr = nc.dram_tensor("kall", [N_CORES, 4, 2, N], f32,
                             addr_space="Shared").ap()

    # matmul m-chunks (PSUM bank <=512 fp32, m-aligned for scatter APs)
    MCH = [(0, 15), (15, 15), (30, 15), (45, 15), (60, 4)]

    with tile.TileContext(nc) as tc:
        with (
            tc.tile_pool(name="cst", bufs=1) as cst,
            tc.tile_pool(name="lda", bufs=1) as lda,
            tc.tile_pool(name="mmp", bufs=4, space="PSUM") as mmp,
            tc.tile_pool(name="nsp", bufs=2, space="PSUM") as nsp,
            tc.tile_pool(name="tp", bufs=2, space="PSUM") as tpp,
            tc.tile_pool(name="gt", bufs=2) as gtp,
            tc.tile_pool(name="sc", bufs=2) as scp,
            tc.tile_pool(name="cf", bufs=2) as cfp,
            tc.tile_pool(name="pde", bufs=1) as pde,
            tc.tile_pool(name="rw", bufs=2) as rwp,
        ):
            # ============ loads & constants ============
            eye = lda.tile([N, N], f32, tag="eye")
            nc.sync.dma_start(eye[:], eye_d[:])
            sel = lda.tile([N, NL], f32, tag="sel")
            nc.sync.dma_start(sel[:], sel_d[:])
            shf = lda.tile([128, N], f32, tag="shf")
            nc.sync.dma_start(shf[:], shf_d[:])
            twoI = cst.tile([N, N], f32, tag="twoI")
            nc.vector.tensor_scalar_mul(twoI[:], eye[:], 2.0)
            ones64 = cst.tile([N, 1], f32, tag="ones64")
            nc.vector.memset(ones64[:], 1.0)
            onesrow = cst.tile([1, N], f32, tag="onesrow")
            nc.vector.memset(onesrow[:], 1.0)

            # B-side (rhs) tensors [D, (m,t)] and A-side (lhsT) [D, (n,s)]
            bt = {}
            for nm, src in (("x", xf), ("y", yf), ("z", zf)):
                t = lda.tile([D, N * S], f32, tag=f"bt_{nm}")
                nc.sync.dma_start(t[:], src.rearrange("m t d -> d (m t)"))
                bt[nm] = t
            at = {}
            for nm, src in (("x", xc), ("y", yc), ("z", zc)):
                t = lda.tile([D, NL * S], f32, tag=f"at_{nm}")
                nc.sync.dma_start(t[:], src.rearrange("n s d -> d (n s)"))
                at[nm] = t

            # ============ x2 (squared norms) for rbf ============
            xsqc = lda.tile([S, NL * D], f32, tag="xsqc")
            nc.sync.dma_start(xsqc[:].rearrange("s (n d) -> s n d", n=NL),
                              xc.rearrange("n s d -> s n d"))
            sqc = lda.tile([S, NL * D], f32, tag="sqc")
            nc.scalar.activation(sqc[:], xsqc[:], AF.Square)
            x2c = lda.tile([S, NL], f32, tag="x2c")
            nc.vector.tensor_reduce(
                x2c[:], sqc[:].rearrange("s (n d) -> s n d", n=NL),
                axis=AX.X, op=OP.add)
            nc.sync.dma_start(x2c_dr.rearrange("n s -> s n"), x2c[:])

            xsqf = lda.tile([S, N * D], f32, tag="xsqf")
            nc.sync.dma_start(xsqf[:].rearrange("t (m d) -> t m d", m=N),
                              xf.rearrange("m t d -> t m d"))
            sqf = lda.tile([S, N * D], f32, tag="sqf")
            nc.scalar.activation(sqf[:], xsqf[:], AF.Square)
            x2f = lda.tile([S, N], f32, tag="x2f")
            nc.vector.tensor_reduce(
                x2f[:], sqf[:].rearrange("t (m d) -> t m d", m=N),
                axis=AX.X, op=OP.add)
            nc.sync.dma_start(x2f_dr.rearrange("m t -> t m"), x2f[:])

            x2B = lda.tile([128, S], f32, tag="x2B")
            for h in range(2):
                nc.sync.dma_start(x2B[h * N:(h + 1) * N, :], x2f_dr[:])

            # inc storage: 16 block slots x (32x32) coarse increments
            incst = pde.tile([128, 16 * CC * CC], f32, tag="incst")

            # ============ gram pipeline ============
            def emit_gram_mms(g, a_nm, b_nm):
                """matmuls + PSUM->DRAM scatter for gram g."""
                for n in range(NL):
                    lhsT = at[a_nm][:, n * S:(n + 1) * S]
                    for (m0, mw) in MCH:
                        ps = mmp.tile([S, 15 * S], f32, tag="mm")
                        w = mw * S
                        nc.tensor.matmul(
                            ps[:, :w], lhsT,
                            bt[b_nm][:, m0 * S:(m0 + mw) * S],
                            start=True, stop=True)
                        st = scp.tile([S, 15 * S], f32, tag="mmst")
                        nc.scalar.copy(st[:, :w], ps[:, :w])
                        nc.sync.dma_start(
                            gdr[g][n, m0:m0 + mw].rearrange("m s t -> s m t"),
                            st[:, :w].rearrange("s (m t) -> s m t", t=S))

            def emit_gram_blocks(g, rbf):
                """gather to pair layout, (rbf assembly), increments."""
                for b in range(4):
                    gt = gtp.tile([128, GRID], f32, tag="gt")
                    nc.sync.dma_start(
                        gt[:],
                        gdr[g][2 * b:2 * b + 2]
                        .rearrange("h m s t -> (h m) (s t)"))
                    if rbf:
                        x2A = scp.tile([128, S], f32, tag="x2A")
                        for h in range(2):
                            nc.sync.dma_start(
                                x2A[h * N:(h + 1) * N, :],
                                x2c_dr[2 * b + h:2 * b + h + 1]
                                .broadcast_to((N, S)))
                        u = scp.tile([128, GRID], f32, tag="u")
                        nc.vector.tensor_tensor(
                            u[:].rearrange("p (s t) -> p s t", t=S),
                            x2A[:].rearrange("p (s o) -> p s o", o=1)
                            .broadcast_to((128, S, S)),
                            x2B[:].rearrange("p (o t) -> p o t", o=1)
                            .broadcast_to((128, S, S)),
                            op=OP.add)
                        v = scp.tile([128, GRID], f32, tag="v")
                        nc.vector.scalar_tensor_tensor(
                            v[:], gt[:], 2.0, u[:],
                            op0=OP.mult, op1=OP.subtract)
                        nc.scalar.activation(gt[:], v[:], AF.Exp)
                    # increments: R = G[:,1:]-G[:,:-1]; inc = R[1:,:]-R[:-1,:]
                    gv = gt[:].rearrange("p (s t) -> p s t", t=S)
                    rt = scp.tile([128, S * (S - 1)], f32, tag="rt")
                    rv = rt[:].rearrange("p (s t) -> p s t", t=S - 1)
                    nc.vector.tensor_tensor(
                        rv, gv[:, :, 1:], gv[:, :, :S - 1], op=OP.subtract)
                    slot = g * 4 + b
                    nc.vector.tensor_tensor(
                        incst[:, slot * CC * CC:(slot + 1) * CC * CC]
                        .rearrange("p (a c) -> p a c", c=CC),
                        rv[:, 1:, :], rv[:, :S - 1, :], op=OP.subtract)

            # ============ PDE row-scan solver ============
            def emit_pde(blk0, nblk, tag):
                W = nblk * 65
                c1s = pde.tile([128, W], f32, tag=f"c1s{tag}")
                dbuf = pde.tile([128, W], f32, tag=f"d{tag}")
                nc.vector.memset(c1s[:], 0.0)
                nc.vector.memset(dbuf[:], 1.0)
                prev = rwp.tile([128, W], f32, tag=f"row{tag}")
                nc.vector.memset(prev[:], 1.0)
                t1 = pde.tile([128, nblk * PQ], f32, tag=f"t1{tag}")
                t2 = pde.tile([128, nblk * PQ], f32, tag=f"t2{tag}")

                inc3 = incst[:].rearrange(
                    "p (k a c) -> p k a c", k=16, a=CC)[:, blk0:blk0 + nblk]

                for r in range(1, PQ + 1):
                    a = (r - 1) // 2
                    if r % 2 == 1:
                        # JIT coefficients for coarse row a
                        inca = inc3[:, :, a, :]            # [128, nblk, 32]
                        s12 = cfp.tile([128, nblk * CC], f32, tag=f"s12{tag}")
                        s12v = s12[:].rearrange("p (b c) -> p b c", c=CC)
                        nc.scalar.activation(s12v, inca, AF.Square,
                                             scale=SQRT192INV)
                        c2r = cfp.tile([128, nblk * CC], f32, tag=f"c2r{tag}")
                        nc.scalar.activation(c2r[:], s12[:], AF.Copy,
                                             scale=-1.0, bias=1.0)
                        vr = cfp.tile([128, nblk * CC], f32, tag=f"vr{tag}")
                        nc.scalar.activation(
                            vr[:].rearrange("p (b c) -> p b c", c=CC),
                            inca, AF.Copy, scale=0.125, bias=1.0)
                        c1r = cfp.tile([128, nblk * CC], f32, tag=f"c1r{tag}")
                        nc.vector.tensor_tensor(c1r[:], s12[:], vr[:], op=OP.add)
                        # stage expanded C1 row (x2 dyadic) into scan coeffs
                        nc.scalar.activation(
                            c1s[:].rearrange("p (b s) -> p b s", s=65)
                            [:, :, 1:65].rearrange("p b (c e) -> p b c e", e=2),
                            c1r[:].rearrange("p (b c o) -> p b c o", c=CC, o=1)
                            .broadcast_to((128, nblk, CC, 2)),
                            AF.Copy)
                        c2b = c2r[:].rearrange("p (b c o) -> p b c o", c=CC, o=1) \
                            .broadcast_to((128, nblk, CC, 2))
                        c1b = c1r[:].rearrange("p (b c o) -> p b c o", c=CC, o=1) \
                            .broadcast_to((128, nblk, CC, 2))

                    pv = prev[:].rearrange("p (b s) -> p b s", s=65)
                    t1v = t1[:].rearrange("p (b s) -> p b s", s=PQ) \
                        .rearrange("p b (c e) -> p b c e", e=2)
                    t2v = t2[:].rearrange("p (b s) -> p b s", s=PQ) \
                        .rearrange("p b (c e) -> p b c e", e=2)
                    nc.vector.tensor_tensor(
                        t1v, pv[:, :, 1:65].rearrange("p b (c e) -> p b c e", e=2),
                        c1b, op=OP.mult)
                    nc.vector.tensor_tensor(
                        t2v, pv[:, :, 0:64].rearrange("p b (c e) -> p b c e", e=2),
                        c2b, op=OP.mult)
                    nc.vector.tensor_tensor(
                        dbuf[:].rearrange("p (b s) -> p b s", s=65)[:, :, 1:65],
                        t1[:].rearrange("p (b s) -> p b s", s=PQ),
                        t2[:].rearrange("p (b s) -> p b s", s=PQ),
                        op=OP.subtract)
                    new = rwp.tile([128, W], f32, tag=f"row{tag}")
                    nc.vector.tensor_tensor_scan(
                        new[:], c1s[:], dbuf[:], 1.0,
                        op0=OP.mult, op1=OP.add)
                    prev = new
                return prev

            # ---- K gram + PDE ----
            emit_gram_mms(0, "x", "x")
            emit_gram_blocks(0, rbf=True)
            lastK = emit_pde(0, 4, "K")
            kvals = cst.tile([128, 4], f32, tag="kvals")
            nc.vector.tensor_copy(
                kvals[:].rearrange("p (b o) -> p b o", o=1),
                lastK[:].rearrange("p (b s) -> p b s", s=65)[:, :, 64:65])
            nc.sync.dma_start(ksh_dr.rearrange("b p -> p b"), kvals[:])
            nc.gpsimd.collective_compute(
                "AllGather", mybir.AluOpType.bypass,
                replica_groups=[list(range(N_CORES))],
                ins=[ksh_dr[:]], outs=[kall_dr[:]])
            kt = cst.tile([N, N], f32, tag="kt")
            nc.sync.dma_start(kt[:], kall_dr.rearrange("c b h m -> (c b h) m"))

            # ---- L grams (overlap with K PDE on other engines) ----
            emit_gram_mms(1, "y", "y")
            emit_gram_mms(2, "z", "z")
            emit_gram_mms(3, "y", "z")
            emit_gram_blocks(1, rbf=False)
            emit_gram_blocks(2, rbf=False)
            emit_gram_blocks(3, rbf=False)

            # ---- Newton-Schulz inverse of M = K + I (replicated) ----
            mt = cst.tile([N, N], f32, tag="mt")
            nc.vector.tensor_tensor(mt[:], kt[:], eye[:], op=OP.add)
            r64 = cst.tile([N, 1], f32, tag="r64")
            nc.vector.tensor_reduce(r64[:], mt[:], axis=AX.X, op=OP.add)
            rT = tpp.tile([1, N], f32, tag="tp")
            nc.tensor.transpose(rT[:], r64[:], eye[:])
            rmax = cst.tile([1, 1], f32, tag="rmax")
            nc.vector.tensor_reduce(rmax[:], rT[:], axis=AX.X, op=OP.max)
            alpha = cst.tile([1, 1], f32, tag="alpha")
            nc.vector.reciprocal(alpha[:], rmax[:])
            alps = tpp.tile([N, 1], f32, tag="tp")
            nc.tensor.matmul(alps[:], onesrow[:], alpha[:], start=True, stop=True)
            alpb = cst.tile([N, 1], f32, tag="alpb")
            nc.scalar.copy(alpb[:], alps[:])
            xns = cst.tile([N, N], f32, tag="xns")
            nc.vector.tensor_scalar_mul(xns[:], eye[:], alpb[:])
            tt = cst.tile([N, N], f32, tag="tt")
            for _ in range(NS_ITERS):
                p1 = nsp.tile([N, N], f32, tag="ns")
                nc.tensor.matmul(p1[:], mt[:], xns[:], start=True, stop=True)
                nc.vector.scalar_tensor_tensor(
                    tt[:], p1[:], -1.0, twoI[:], op0=OP.mult, op1=OP.add)
                p2 = nsp.tile([N, N], f32, tag="ns")
                nc.tensor.matmul(p2[:], xns[:], tt[:], start=True, stop=True)
                nc.scalar.copy(xns[:], p2[:])
            # E = B - B^2
            p3 = nsp.tile([N, N], f32, tag="ns")
            nc.tensor.matmul(p3[:], xns[:], xns[:], start=True, stop=True)
            # et = xns - p3 : (p3 * -1) + xns
            et = cst.tile([N, N], f32, tag="et")
            nc.vector.scalar_tensor_tensor(
                et[:], p3[:], -1.0, xns[:], op0=OP.mult, op1=OP.add)
            ecp = nsp.tile([N, NL], f32, tag="ns")
            nc.tensor.matmul(ecp[:], et[:], sel[:], start=True, stop=True)
            ecols = cst.tile([N, NL], f32, tag="ecols")
            nc.scalar.copy(ecols[:], ecp[:])

            # ---- L PDE + partial trace ----
            lastL = emit_pde(4, 12, "L")
            lvals = cst.tile([128, 12], f32, tag="lvals")
            nc.vector.tensor_copy(
                lvals[:].rearrange("p (b o) -> p b o", o=1),
                lastL[:].rearrange("p (b s) -> p b s", s=65)[:, :, 64:65])
            lsum = cst.tile([128, 4], f32, tag="lsum")
            nc.vector.tensor_tensor(lsum[:], lvals[:, 0:4], lvals[:, 4:8],
                                    op=OP.add)
            nc.vector.scalar_tensor_tensor(
                lsum[:], lvals[:, 8:12], -2.0, lsum[:], op0=OP.mult, op1=OP.add)
            lup_p = tpp.tile([N, 4], f32, tag="tp")
            nc.tensor.matmul(lup_p[:], shf[:], lsum[:], start=True, stop=True)
            lup = cst.tile([N, 4], f32, tag="lup")
            nc.scalar.copy(lup[:], lup_p[:])
            prodA = cst.tile([N, 4], f32, tag="prodA")
            nc.vector.tensor_tensor(
                prodA[:], lsum[0:N, :],
                ecols[:].rearrange("p (c e) -> p c e", e=2)[:, :, 0],
                op=OP.mult)
            prodB = cst.tile([N, 4], f32, tag="prodB")
            nc.vector.tensor_tensor(
                prodB[:], lup[:],
                ecols[:].rearrange("p (c e) -> p c e", e=2)[:, :, 1],
                op=OP.mult)
            ra = cst.tile([N, 1], f32, tag="ra")
            nc.vector.tensor_reduce(ra[:], prodA[:], axis=AX.X, op=OP.add)
            rb = cst.tile([N, 1], f32, tag="rb")
            nc.vector.tensor_reduce(rb[:], prodB[:], axis=AX.X, op=OP.add)
            vsum = cst.tile([N, 1], f32, tag="vsum")
            nc.vector.tensor_tensor(vsum[:], ra[:], rb[:], op=OP.add)
            part = tpp.tile([1, 1], f32, tag="tp")
            nc.tensor.matmul(part[:], vsum[:], ones64[:], start=True, stop=True)
            outst = cst.tile([1, 1], f32, tag="outst")
            nc.scalar.copy(outst[:], part[:])
            nc.sync.dma_start(out_d[:], outst[:])

    nc.compile()
    return nc


def _host_inputs(x, y, z):
    eye = np.eye(N, dtype=np.float32)
    shf = np.zeros((128, N), dtype=np.float32)
    for p in range(N):
        shf[p + N, p] = 1.0
    maps = []
    for c in range(N_CORES):
        sel = np.zeros((N, NL), dtype=np.float32)
        for j in range(NL):
            sel[NL * c + j, j] = 1.0
        maps.append({
            "xf": np.ascontiguousarray(x), "yf": np.ascontiguousarray(y),
            "zf": np.ascontiguousarray(z),
            "xc": np.ascontiguousarray(x[NL * c:NL * (c + 1)]),
            "yc": np.ascontiguousarray(y[NL * c:NL * (c + 1)]),
            "zc": np.ascontiguousarray(z[NL * c:NL * (c + 1)]),
            "eye": eye, "sel": sel, "shf": shf,
        })
    return maps


def _make_runtime():
    """Build the Bass module once and wrap it in a cached jitted callable.

    Per call only x,y,z move host->device (one packed buffer); the
    replication to per-core full copies, the per-core row slices, and the
    donated zero output buffer are all produced on-device by a prep jit.
    Constant inputs (eye/sel/shf) stay resident across calls.
    """
    import jax
    import jax.numpy as jnp
    from jax.experimental.shard_map import shard_map
    from jax.sharding import Mesh, NamedSharding, PartitionSpec
    from concourse import bass2jax, mybir

    nc = _build()
    bass2jax.install_neuronx_cc_hook()

    partition_name = nc.partition_id_tensor.name if nc.partition_id_tensor else None
    in_names, out_names, out_avals = [], [], []
    for alloc in nc.m.functions[0].allocations:
        if not isinstance(alloc, mybir.MemoryLocationSet):
            continue
        name = alloc.memorylocations[0].name
        if alloc.kind == "ExternalInput":
            if name != partition_name:
                in_names.append(name)
        elif alloc.kind == "ExternalOutput":
            out_names.append(name)
            out_avals.append(jax.core.ShapedArray(
                tuple(alloc.tensor_shape), mybir.dt.np(alloc.dtype)))
    n_params = len(in_names)
    in_names_full = in_names + out_names + (
        [partition_name] if partition_name else [])
    donate = tuple(range(n_params, n_params + len(out_names)))

    def _body(*args):
        operands = list(args)
        if partition_name is not None:
            operands.append(bass2jax.partition_id_tensor())
        return tuple(bass2jax._bass_exec_p.bind(
            *operands, out_avals=tuple(out_avals),
            in_names=tuple(in_names_full), out_names=tuple(out_names),
            lowering_input_output_aliases=(),
            sim_require_finite=True, sim_require_nnan=True, nc=nc))

    devices = jax.devices()[:N_CORES]
    mesh = Mesh(np.asarray(devices), ("core",))
    shard = NamedSharding(mesh, PartitionSpec("core"))
    repl = NamedSharding(mesh, PartitionSpec())
    in_specs = (PartitionSpec("core"),) * (n_params + len(out_names))
    out_specs = (PartitionSpec("core"),) * len(out_names)
    sharded = jax.jit(
        shard_map(_body, mesh=mesh, in_specs=in_specs, out_specs=out_specs,
                  check_rep=False),
        donate_argnums=donate, keep_unused=True)

    # constants, staged device-resident once (global = per-core concat)
    eye = np.eye(N, dtype=np.float32)
    shf = np.zeros((128, N), dtype=np.float32)
    for p in range(N):
        shf[p + N, p] = 1.0
    sel_g = np.zeros((N_CORES, N, NL), np.float32)
    for c in range(N_CORES):
        for j in range(NL):
            sel_g[c, NL * c + j, j] = 1.0
    const = {
        "eye": jax.device_put(np.tile(eye, (N_CORES, 1)), shard),
        "sel": jax.device_put(sel_g.reshape(N_CORES * N, NL), shard),
        "shf": jax.device_put(np.tile(shf, (N_CORES, 1)), shard),
    }
    jax.block_until_ready(list(const.values()))

    def _prep(packed):
        x, y, z = packed[0], packed[1], packed[2]
        xf = jnp.tile(x, (N_CORES, 1, 1))
        yf = jnp.tile(y, (N_CORES, 1, 1))
        zf = jnp.tile(z, (N_CORES, 1, 1))
        zer = jnp.zeros((N_CORES, 1, 1), jnp.float32)
        return xf, yf, zf, x, y, z, zer

    prep = jax.jit(_prep, in_shardings=(repl,), out_shardings=(shard,) * 7)
    name2idx = {nm: i for i, nm in enumerate(in_names)}

    def call(x, y, z):
        packed = np.stack([x, y, z])
        xf, yf, zf, xc, yc, zc, zer = prep(packed)
        args = [None] * n_params
        args[name2idx["xf"]] = xf
        args[name2idx["yf"]] = yf
        args[name2idx["zf"]] = zf
        args[name2idx["xc"]] = xc
        args[name2idx["yc"]] = yc
        args[name2idx["zc"]] = zc
        for nm, buf in const.items():
            args[name2idx[nm]] = buf
        out = sharded(*args, zer)
        vals = np.asarray(out[0]).reshape(N_CORES)
        return np.float32(np.float64(vals).sum())

    return call


def _kernel_fallback(x, y, z):
    from concourse import bass_utils
    if "nc" not in _CACHE:
        _CACHE["nc"] = _build()
    nc = _CACHE["nc"]
    maps = _host_inputs(x, y, z)
    res = bass_utils.run_bass_kernel_spmd(nc, maps, core_ids=list(range(N_CORES)))
    total = np.float64(0.0)
    for c in range(N_CORES):
        total += np.float64(res.results[c]["out"][0, 0])
    return np.float32(total)


def kernel(x, y, z):
    x = np.ascontiguousarray(np.asarray(x, np.float32))
    y = np.ascontiguousarray(np.asarray(y, np.float32))
    z = np.ascontiguousarray(np.asarray(z, np.float32))
    if not _CACHE.get("rt_failed"):
        # retry once with a fresh runtime: a transient device error
        # (e.g. NRT_EXEC_UNIT_UNRECOVERABLE) usually clears on re-run
        for _ in range(2):
            try:
                if "rt" not in _CACHE:
                    _CACHE["rt"] = _make_runtime()
                return _CACHE["rt"](x, y, z)
            except Exception:
                _CACHE.pop("rt", None)
        _CACHE["rt_failed"] = True
    return _kernel_fallback(x, y, z)

